# revision 8
# baseline (speedup 1.0000x reference)
"""GAT layer (N=8192, F_in=256, F_out=64) on 8 trn2 NeuronCores.

Strategy (rows of adj sharded across 8 cores, W/a replicated):
  reference:  h = X@W;  e = leakyrelu(s1[i]+s2[j]);  att = softmax(mask(e));
              out = elu(att @ h)
  identity used:  exp(leakyrelu(x)) = max(exp(x), exp(0.2 x))   (exp monotone)
  so the big [N,N] stage needs NO transcendentals:
      p[j,i] = min( max( u2[j]*u1[i], exp(0.2 s1[i] + 0.2 s2[j]) ), mask[j,i] )
  with mask in {0, 32768} (host-prepped, fp16, TRANSPOSED so the contraction
  index j lands on SBUF partitions), u = exp(s).  The mask multiply is exact:
  min(positive, 0) = 0, min(p, 32768) = p.  It is applied for free inside the
  mask-load DMA via the SDMA inline-ALU (accum_op=min).
  Then  h'T[f,i] (+rowsum as a 65th row) = sum_jb hplus[jb].T @ p[jb]  on PE,
  normalize + ELU on the tiny [1024, 64] result.
"""

import os
import sys

for _p in ("/opt/trn_rl_repo", "/root/.axon_site", "/root/.axon_site/_ro/trn_rl_repo"):
    if os.path.isdir(_p) and _p not in sys.path:
        sys.path.append(_p)

import numpy as np
from contextlib import ExitStack

import concourse.bass as bass
import concourse.bacc as bacc
import concourse.tile as tile
from concourse import mybir
from concourse.bass_utils import run_bass_kernel_spmd

N = 8192
FI = 256
FO = 64
NCORES = 8
IOWN = N // NCORES          # 1024 rows of output per core
NB = N // 128               # 64 j-blocks
NC_CHUNK = 512              # prologue i-chunk width
F32 = mybir.dt.float32
F16 = mybir.dt.float16

MASK_NEG = np.float16(-32768.0)   # additive mask: p = relu(max(t1,t2) + mask)

# Set to False to apply the mask with DVE tensor ops instead of the
# SDMA inline-add (fallback if accum_op misbehaves).
USE_DMA_ACCUM_MASK = True


def build_program():
    nc = bacc.Bacc("TRN2", target_bir_lowering=False, debug=False, num_devices=NCORES)

    xt = nc.dram_tensor("xt", [FI, N], F32, kind="ExternalInput").ap()
    xto = nc.dram_tensor("xto", [FI, IOWN], F32, kind="ExternalInput").ap()
    maskt = nc.dram_tensor("maskt", [N, IOWN], F16, kind="ExternalInput").ap()
    w = nc.dram_tensor("w", [FI, FO], F32, kind="ExternalInput").ap()
    acat = nc.dram_tensor("acat", [FO, 2], F32, kind="ExternalInput").ap()
    out = nc.dram_tensor("out", [IOWN, FO], F32, kind="ExternalOutput").ap()

    with tile.TileContext(nc) as tc:
        with ExitStack() as ctx:
            gat_body(ctx, tc, out, xt, xto, maskt, w, acat)
    nc.finalize()
    return nc


def gat_body(ctx, tc, out, xt, xto, maskt, w, acat):
    nc = tc.nc
    ACT = mybir.ActivationFunctionType
    ALU = mybir.AluOpType

    xt_r = xt.rearrange("(k p) n -> k p n", p=128)        # [2,128,N]
    xto_r = xto.rearrange("(k p) n -> k p n", p=128)      # [2,128,IOWN]
    maskt_r = maskt.rearrange("(b p) i -> b p i", p=128)  # [NB,128,IOWN]
    out_r = out.rearrange("(s p) f -> s p f", p=128)      # [8,128,FO]

    cp = ctx.enter_context(tc.tile_pool(name="const", bufs=1))
    xp = ctx.enter_context(tc.tile_pool(name="xload", bufs=3))
    hp = ctx.enter_context(tc.tile_pool(name="htchunk", bufs=2))
    psw = ctx.enter_context(tc.tile_pool(name="psw", bufs=2, space="PSUM"))
    acc = ctx.enter_context(tc.tile_pool(name="acc", bufs=1, space="PSUM"))
    t1p = ctx.enter_context(tc.tile_pool(name="t1", bufs=3))
    t2p = ctx.enter_context(tc.tile_pool(name="t2", bufs=3))
    pp = ctx.enter_context(tc.tile_pool(name="p", bufs=4))
    ep = ctx.enter_context(tc.tile_pool(name="epi", bufs=2))

    # ---- persistent SBUF ----
    w_sb = cp.tile([128, 2, FO], F32)          # W k-tiles (lhsT for hT)
    acat_sb = cp.tile([FO, 2], F32)            # [a1 | a2] (lhsT for s)
    ones1 = cp.tile([1, 128], F32)             # k=1 broadcast lhsT
    hplus = cp.tile([128, NB, FO + 1], F16)    # h rows + ones col (lhsT, per jb)
    u1b = cp.tile([128, IOWN], F16)            # exp(s1[i]) broadcast over partitions
    sb02 = cp.tile([128, IOWN], F16)           # 0.2*s1[i]  broadcast over partitions
    u2col = cp.tile([128, NB], F32)            # exp(s2[j]), j = jb*128 + p
    s2col = cp.tile([128, NB], F32)            # s2[j] folded to column layout
    s2col02 = cp.tile([128, NB], F32)          # 0.2*s2[j]
    so_sb = cp.tile([1, IOWN], F32)            # s1 of own rows
    hto_sb = cp.tile([FO, IOWN], F32)          # hT of own rows
    iota_t = cp.tile([128, 128], mybir.dt.int32)
    ident = cp.tile([128, 128], F32)

    for k in range(2):
        nc.sync.dma_start(w_sb[:, k, :], w.rearrange("(k p) f -> k p f", p=128)[k])
    nc.sync.dma_start(acat_sb[:, :], acat[:, :])
    nc.vector.memset(ones1[:, :], 1.0)
    nc.vector.memset(hplus[:, :, FO : FO + 1], 1.0)
    nc.gpsimd.iota(iota_t[:, :], pattern=[[1, 128]], base=0, channel_multiplier=-1)
    nc.vector.tensor_scalar(ident[:, :], iota_t[:, :], 0, None, op0=ALU.is_equal)

    # ---- own-rows path: s1_own, exp broadcasts ----
    for cc in range(2):
        sl = slice(cc * NC_CHUNK, (cc + 1) * NC_CHUNK)
        xo = xp.tile([128, 2, NC_CHUNK], F32, tag="xt")
        for k in range(2):
            nc.sync.dma_start(xo[:, k, :], xto_r[k, :, sl])
        hto_ps = psw.tile([FO, NC_CHUNK], F32, tag="psw")
        nc.tensor.matmul(hto_ps[:, :], w_sb[:, 0, :], xo[:, 0, :], start=True, stop=False)
        nc.tensor.matmul(hto_ps[:, :], w_sb[:, 1, :], xo[:, 1, :], start=False, stop=True)
        nc.scalar.activation(hto_sb[:, sl], hto_ps[:, :], ACT.Copy)
        so_ps = psw.tile([1, NC_CHUNK], F32, tag="psw")
        nc.tensor.matmul(so_ps[:, :], acat_sb[:, 0:1], hto_sb[:, sl], start=True, stop=True)
        nc.vector.tensor_copy(so_sb[:, sl], so_ps[:, :])
        bc_ps = psw.tile([128, NC_CHUNK], F32, tag="psw")
        nc.tensor.matmul(bc_ps[:, :], ones1[:, :], so_sb[0:1, sl], start=True, stop=True)
        nc.scalar.activation(u1b[:, sl], bc_ps[:, :], ACT.Exp)
        nc.scalar.activation(sb02[:, sl], bc_ps[:, :], ACT.Copy, scale=0.2)

    # ---- full-graph prologue: h (natural + transposed), s2 ----
    for c in range(N // NC_CHUNK):
        sl = slice(c * NC_CHUNK, (c + 1) * NC_CHUNK)
        xc = xp.tile([128, 2, NC_CHUNK], F32, tag="xt")
        for k in range(2):
            nc.sync.dma_start(xc[:, k, :], xt_r[k, :, sl])
        # hT chunk (for s2)
        ht_ps = psw.tile([FO, NC_CHUNK], F32, tag="psw")
        nc.tensor.matmul(ht_ps[:, :], w_sb[:, 0, :], xc[:, 0, :], start=True, stop=False)
        nc.tensor.matmul(ht_ps[:, :], w_sb[:, 1, :], xc[:, 1, :], start=False, stop=True)
        ht_sb = hp.tile([FO, NC_CHUNK], F32)
        nc.scalar.activation(ht_sb[:, :], ht_ps[:, :], ACT.Copy)
        # s2 directly in column layout: out[j_sub, b] = sum_f hT[f, j] * a2[f]
        bsl = slice(c * 4, c * 4 + 4)
        s_ps = psw.tile([128, 4], F32, tag="psw")
        for b in range(4):
            nc.tensor.matmul(
                s_ps[:, b : b + 1],
                ht_sb[:, b * 128 : (b + 1) * 128],
                acat_sb[:, 1:2],
                start=True, stop=True,
            )
        nc.vector.tensor_copy(s2col[:, bsl], s_ps[:, :])
        nc.scalar.activation(u2col[:, bsl], s2col[:, bsl], ACT.Exp)
        nc.vector.tensor_scalar_mul(s2col02[:, bsl], s2col[:, bsl], 0.2)
        # h natural rows c*512 .. (for hplus lhsT)
        h_ps = psw.tile([128, 4, FO], F32, tag="psw")
        for m in range(4):
            nc.tensor.matmul(
                h_ps[:, m, :], xc[:, 0, m * 128 : (m + 1) * 128], w_sb[:, 0, :],
                start=True, stop=False,
            )
            nc.tensor.matmul(
                h_ps[:, m, :], xc[:, 1, m * 128 : (m + 1) * 128], w_sb[:, 1, :],
                start=False, stop=True,
            )
        nc.vector.tensor_copy(hplus[:, bsl, 0:FO], h_ps[:, :, :])

    # ---- hot loop over j-blocks ----
    acc0 = acc.tile([FO + 1, 512], F32, tag="acc0")
    acc1 = acc.tile([FO + 1, 512], F32, tag="acc1")
    for jb in range(NB):
        t1 = t1p.tile([128, IOWN], F16)
        nc.vector.tensor_scalar_mul(t1[:, :], u1b[:, :], u2col[:, jb : jb + 1])
        t2 = t2p.tile([128, IOWN], F16)
        nc.scalar.activation(
            t2[:, :], sb02[:, :], ACT.Exp, bias=s2col02[:, jb : jb + 1]
        )
        pm = pp.tile([128, IOWN], F16, tag="pm")
        nc.vector.tensor_max(pm[:, :], t1[:, :], t2[:, :])
        p = pp.tile([128, IOWN], F16, tag="p")
        if USE_DMA_ACCUM_MASK:
            nc.gpsimd.dma_start(pm[:, :], maskt_r[jb], accum_op=mybir.AluOpType.add)
            nc.gpsimd.tensor_scalar_max(p[:, :], pm[:, :], 0.0)
        else:
            m = t1p.tile([128, IOWN], F16, tag="mload")
            nc.sync.dma_start(m[:, :], maskt_r[jb])
            nc.vector.tensor_tensor(pm[:, :], pm[:, :], m[:, :], op=mybir.AluOpType.add)
            nc.vector.tensor_scalar_max(p[:, :], pm[:, :], 0.0)
        nc.tensor.matmul(
            acc0[:, :], hplus[:, jb, :], p[:, 0:512],
            start=(jb == 0), stop=(jb == NB - 1),
        )
        nc.tensor.matmul(
            acc1[:, :], hplus[:, jb, :], p[:, 512:1024],
            start=(jb == 0), stop=(jb == NB - 1),
        )

    # ---- epilogue: transpose, normalize, ELU, store ----
    acc_sb = ep.tile([FO + 1, IOWN], F32, tag="accsb")
    nc.scalar.activation(acc_sb[:, 0:512], acc0[:, :], ACT.Copy)
    nc.scalar.activation(acc_sb[:, 512:1024], acc1[:, :], ACT.Copy)
    for s in range(IOWN // 128):
        ssl = slice(s * 128, (s + 1) * 128)
        tr_ps = psw.tile([128, FO + 1], F32, tag="psw")
        nc.tensor.transpose(tr_ps[:, :], acc_sb[:, ssl], ident[0 : FO + 1, 0 : FO + 1])
        rcol = ep.tile([128, 1], F32, tag="rcol")
        nc.vector.reciprocal(rcol[:, :], tr_ps[:, FO : FO + 1])
        nrm = ep.tile([128, FO], F32, tag="nrm")
        nc.vector.tensor_scalar_mul(nrm[:, :], tr_ps[:, 0:FO], rcol[:, :])
        xneg = ep.tile([128, FO], F32, tag="xneg")
        nc.vector.tensor_scalar_min(xneg[:, :], nrm[:, :], 0.0)
        eneg = ep.tile([128, FO], F32, tag="eneg")
        nc.scalar.activation(eneg[:, :], xneg[:, :], ACT.Exp)
        xpos = ep.tile([128, FO], F32, tag="xpos")
        nc.vector.tensor_scalar_max(xpos[:, :], nrm[:, :], 0.0)
        esum = ep.tile([128, FO], F32, tag="esum")
        nc.vector.tensor_add(esum[:, :], xpos[:, :], eneg[:, :])
        fin2 = ep.tile([128, FO], F32, tag="fin2")
        nc.vector.tensor_scalar_add(fin2[:, :], esum[:, :], -1.0)
        nc.sync.dma_start(out_r[s], fin2[:, :])


_prog_cache = {}


def _get_program():
    if "nc" not in _prog_cache:
        _prog_cache["nc"] = build_program()
    return _prog_cache["nc"]


def _host_prep(input, adj, W, a):
    x = np.ascontiguousarray(np.asarray(input, dtype=np.float32))
    adj = np.asarray(adj)
    W = np.ascontiguousarray(np.asarray(W, dtype=np.float32))
    a = np.asarray(a, dtype=np.float32)

    xt_full = np.ascontiguousarray(x.T)                      # [FI, N]
    acat = np.ascontiguousarray(
        np.stack([a[:FO, 0], a[FO:, 0]], axis=1)             # [FO, 2]
    )
    in_maps = []
    for c in range(NCORES):
        rows = slice(c * IOWN, (c + 1) * IOWN)
        maskt = np.where(
            adj[rows, :].T > 0, np.float16(0.0), MASK_NEG
        ).astype(np.float16)                                 # [N, IOWN]
        maskt = np.ascontiguousarray(maskt)
        xto = np.ascontiguousarray(xt_full[:, rows])         # [FI, IOWN]
        in_maps.append(
            {"xt": xt_full, "xto": xto, "maskt": maskt, "w": W, "acat": acat}
        )
    return in_maps


def run(input, adj, W, a, trace=False, **trace_kwargs):
    nc = _get_program()
    in_maps = _host_prep(input, adj, W, a)
    res = run_bass_kernel_spmd(
        nc, in_maps, core_ids=list(range(NCORES)), trace=trace, **trace_kwargs
    )
    outp = np.concatenate([r["out"] for r in res.results], axis=0)
    return outp, res


def kernel(input, adj, W, a):
    outp, _ = run(input, adj, W, a, trace=False)
    return outp
